# revision 32
# baseline (speedup 1.0000x reference)
"""MultiHeadAttention Trainium2 kernel (v3).

B=2, S=2048, E=1024, H=16, D=64. 8 NeuronCores.

Sharding: B*H = 32 (batch, head) pairs -> 4 heads per core (core c handles
batch c//4, heads 4*(c%4)..4*(c%4)+3). Out-projection is column-sharded by
head (Wo folded with Wv); partial [S, E] outputs are summed on host (the
"all-reduce"), each core adding bo/4 so the sum carries the bias exactly once.

Math (per head h):
  S_scores = (q @ Wq.T) @ (k @ Wk.T).T / sqrt(D)  ==  q @ (A/8) @ k.T,
    A = Wq.T @ Wk  (so q needs no projection on device)
  P = softmax(mask(S_scores))  (unnormalized exp + ones-column trick)
  ctx = P @ v  (raw v; Wv folded into Wo)
  out_h = ctx @ (Wo[:, cols_h] @ Wv).T

v3 schedule: one work item = one (sq-chunk, head-pair, sk-block). Its score
tile [128, 2*CHUNK] holds BOTH heads of the pair side by side, so a single
Activation instruction exps both heads, and one item consumes one PSUM
buffer (bufs=2 gives one item of lookahead). Items stream flat across
pairs and chunks with scores emitted one item ahead of the exp->ctx
consumers; kA for chunk ci+1 is hoisted into chunk ci; the chunk's output
projection is deferred and drained between the next chunk's items to fill
PE gaps. The q/k score path runs in bf16 (fp32r has an N>=256 fast-path
constraint; bf16 allows exact causal column starts), the v/es ctx path in
bf16, and the out-projection in fp32r. Causal masking is a [128,128]
triangle bf16 multiply on DVE; exp is the only Activation-engine op.
"""

import sys

if "/opt/trn_rl_repo" not in sys.path:
    sys.path.insert(0, "/opt/trn_rl_repo")

import os

import numpy as np

import concourse.bass as bass
import concourse.tile as tile
from concourse import bacc, mybir
from concourse.bass_utils import run_bass_kernel_spmd

K_KA_POOL = os.environ.get("K_KA_POOL", "sp")  # sp | op
K_BCAST = os.environ.get("K_BCAST", "pool")  # pe | pool

B, S, E, H = 2, 2048, 1024, 16
D = E // H  # 64
N_CORES = 8
HEADS_PER_CORE = H * B // N_CORES  # 4
N_CHUNK = 4  # sq chunks of 512
CHUNK = S // N_CHUNK  # 512
N_BLK = S // 128  # 16 sk blocks of 128
F32 = mybir.dt.float32
F32R = mybir.dt.float32r
BF16 = mybir.dt.bfloat16
BF16_NP = mybir.dt.np(mybir.dt.bfloat16)


def _analyze_mask(mask):
    """Classify each (sq-chunk, sk-block) region of the shared mask.

    Returns (schedule, tiles): schedule[ci] is a list of (blk, mode, aux)
    with mode in {"plain", "causal", "tile"}; tiles is the list of distinct
    float32 [128, CHUNK] (sk, sq) multiplicative mask tiles for "tile" mode.
    """
    m = np.asarray(mask).reshape(S, S) != 0
    schedule = []
    tiles = []
    tile_index = {}
    for ci in range(N_CHUNK):
        q0 = ci * CHUNK
        blks = []
        for k in range(N_BLK):
            k0 = k * 128
            mb = m[q0 : q0 + CHUNK, k0 : k0 + 128]  # [sq, sk]
            if not mb.any():
                continue
            if mb.all():
                blks.append((k, "plain", None))
                continue
            causal = (
                np.arange(q0, q0 + CHUNK)[:, None] >= np.arange(k0, k0 + 128)[None, :]
            )
            if np.array_equal(mb, causal):
                blks.append((k, "causal", None))
            else:
                t = np.ascontiguousarray(mb.T.astype(np.float32))  # [sk, sq]
                key = t.tobytes()
                if key not in tile_index:
                    tile_index[key] = len(tiles)
                    tiles.append(t)
                blks.append((k, "tile", tile_index[key]))
        schedule.append(blks)
    return schedule, tiles


def build_nc(schedule, n_mask_tiles, repeat=1, hw_loop=0):
    """Build the SPMD Bass program (identical for all 8 cores)."""
    nc = bacc.Bacc(
        "TRN2", target_bir_lowering=False, debug=False, num_devices=N_CORES
    )

    qT_d = nc.dram_tensor("qT", [2, 128, S], BF16, kind="ExternalInput").ap()
    kT_d = nc.dram_tensor("kT", [2, 128, S], BF16, kind="ExternalInput").ap()
    va_d = nc.dram_tensor(
        "va", [4, 128, N_BLK * (D + 1)], BF16, kind="ExternalInput"
    ).ap()
    tri_d = nc.dram_tensor("tri", [128, 128], BF16, kind="ExternalInput").ap()
    wq_d = nc.dram_tensor("wq", [D, D], F32, kind="ExternalInput").ap()
    wk_d = nc.dram_tensor("wk", [D, D], F32, kind="ExternalInput").ap()
    wv_d = nc.dram_tensor("wv", [D, D], F32, kind="ExternalInput").ap()
    woT_d = nc.dram_tensor("woT", [4, D, E], F32, kind="ExternalInput").ap()
    bo4_d = nc.dram_tensor("bo4", [1, E], F32, kind="ExternalInput").ap()
    if n_mask_tiles:
        mt_d = nc.dram_tensor(
            "mtiles", [n_mask_tiles, 128, CHUNK], BF16, kind="ExternalInput"
        ).ap()
    out_d = nc.dram_tensor("out", [S, E], BF16, kind="ExternalOutput").ap()

    Exp = mybir.ActivationFunctionType.Exp

    from contextlib import ExitStack

    with tile.TileContext(nc) as tc, ExitStack() as ctx:
        const = ctx.enter_context(tc.tile_pool(name="const", bufs=1))
        qk = ctx.enter_context(tc.tile_pool(name="qk", bufs=1))
        va_pool = ctx.enter_context(tc.tile_pool(name="vap", bufs=1))
        es_pool = ctx.enter_context(tc.tile_pool(name="es", bufs=8))
        ctxn = ctx.enter_context(tc.tile_pool(name="ctxn", bufs=6))
        rr = ctx.enter_context(tc.tile_pool(name="rr", bufs=4))
        outp = ctx.enter_context(tc.tile_pool(name="outp", bufs=3))
        sp = ctx.enter_context(tc.tile_pool(name="sp", bufs=2, space="PSUM"))
        cp = ctx.enter_context(tc.tile_pool(name="cp", bufs=2, space="PSUM"))
        op = ctx.enter_context(tc.tile_pool(name="op", bufs=2, space="PSUM"))

        # ---- constants / weight prep ----
        wq_sb = const.tile([D, D], F32, tag="wq")
        # Wk loaded twice side by side: the A.T matmul then yields A.T
        # replicated on partitions 0-63 and 64-127 in one shot
        wk2_sb = const.tile([D, 2 * D], F32, tag="wk2")
        wv_sb = const.tile([D, D], F32R, tag="wv")
        nc.sync.dma_start(wq_sb[:], wq_d[:])
        nc.sync.dma_start(wk2_sb[:, 0:D], wk_d[:])
        nc.sync.dma_start(wk2_sb[:, D : 2 * D], wk_d[:])
        nc.sync.dma_start(wv_sb[:], wv_d[:].bitcast(F32R))

        # A.T/8 = (Wk.T @ Wq)/8  [d', d], replicated over both partition halves
        at_ps = sp.tile([128, D], F32, tag="scores")
        nc.tensor.matmul(at_ps[:], wk2_sb[:], wq_sb[:], start=True, stop=True)
        at_sb = const.tile([128, D], BF16, tag="at")
        nc.vector.tensor_scalar_mul(at_sb[:], at_ps[:], 1.0 / np.sqrt(float(D)))
        # ones row for the K=1 broadcast matmul (r_inv -> r_bc in PSUM)
        ones_sb = None
        if K_BCAST == "pe":
            ones_sb = const.tile([1, D], F32R, tag="ones")
            nc.vector.memset(ones_sb[:], 1.0)

        wovT, mtiles = [], []
        bo4_bc = None
        tri_sb = const.tile([128, 128], BF16, tag="tri")

        def _emit_prep():
            nonlocal bo4_bc
            nc.sync.dma_start(tri_sb[:], tri_d[:])
            # dummy exp so the Exp activation table loads during prep, not
            # inside the hw loop body
            warm = const.tile([1, 1], F32, tag="warm")
            nc.scalar.activation(warm[:], wq_sb[0:1, 0:1], Exp)
            for p in range(2):
                wovT_p = const.tile([128, E], F32R, tag=f"wovT{p}", name=f"wovT{p}")
                wovT.append(wovT_p)
            for h in range(4):
                woT_sb = const.tile([D, E], F32R, tag="woT_ld")
                nc.sync.dma_start(woT_sb[:], woT_d[h].bitcast(F32R))
                p, o = h // 2, (h % 2) * D
                for ec in range(E // 512):
                    wo_ps = op.tile([D, 512], F32, tag="ctx")
                    nc.tensor.matmul(
                        wo_ps[:],
                        wv_sb[:],
                        woT_sb[:, ec * 512 : (ec + 1) * 512],
                        start=True,
                        stop=True,
                    )
                    nc.vector.tensor_copy(
                        wovT[p][o : o + D, ec * 512 : (ec + 1) * 512], wo_ps[:]
                    )
            bo4_sb = const.tile([1, E], F32, tag="bo4")
            nc.sync.dma_start(bo4_sb[:], bo4_d[:])
            bo4_bc = const.tile([128, E], F32, tag="bo4bc")
            nc.gpsimd.partition_broadcast(bo4_bc[:], bo4_sb[:])
            for i in range(n_mask_tiles):
                t = const.tile([128, CHUNK], BF16, tag=f"mt{i}", name=f"mt{i}")
                nc.sync.dma_start(t[:], mt_d[i])
                mtiles.append(t)

        def _emit_body(_first):
            # ---- input loads, ci-major so the pipeline can start early ----
            qT = []
            kAT = []
            va = []
            k_sb_l = []
            for p in range(2):
                qT.append(qk.tile([128, S], BF16, tag=f"qT{p}", name=f"qT{p}"))
                k_sb_l.append(qk.tile([128, S], BF16, tag=f"kT{p}", name=f"kT{p}"))
                kAT.append(qk.tile([128, S], BF16, tag=f"kAT{p}", name=f"kAT{p}"))
            for h in range(4):
                va.append(
                    va_pool.tile(
                        [128, N_BLK * (D + 1)], BF16, tag=f"va{h}", name=f"va{h}"
                    )
                )
            chunk_order = sorted(
                range(N_CHUNK), key=lambda ci: -len(schedule[ci])
            )
            for oi, ci in enumerate(chunk_order):
                cs = slice(ci * CHUNK, (ci + 1) * CHUNK)
                for p in range(2):
                    nc.sync.dma_start(k_sb_l[p][:, cs], kT_d[p, :, cs])
                    nc.gpsimd.dma_start(qT[p][:, cs], qT_d[p, :, cs])
                if oi < 2:
                    for hh in range(2):
                        h = 2 * oi + hh
                        nc.gpsimd.dma_start(va[h][:], va_d[h])

            if _first and not hw_loop:
                _emit_prep()

            def emit_kA(ci):
                # all 4 (pair, head) kA matmuls land in ONE score-pool tile
                # (disjoint partition/column quadrants), so the hoisted kA
                # costs a single PSUM slot-wait and two gpsimd copies.
                cs_k = slice(ci * CHUNK, (ci + 1) * CHUNK)
                ka_ps = sp.tile([128, 2 * CHUNK], F32, tag="scores", name="ka_ps")
                for p_ in range(2):
                    for hh in range(2):
                        o = hh * D
                        nc.tensor.matmul(
                            ka_ps[o : o + D, p_ * CHUNK : (p_ + 1) * CHUNK],
                            at_sb[o : o + D, :],
                            k_sb_l[p_][o : o + D, cs_k],
                            start=True,
                            stop=True,
                        )
                # gpsimd cannot read PSUM; these copies must ride DVE
                for p_ in range(2):
                    nc.vector.tensor_copy(
                        kAT[p_][:, cs_k], ka_ps[:, p_ * CHUNK : (p_ + 1) * CHUNK]
                    )

            # ---- deferred output-projection half-units (one per (sb, ec)) ----
            pending = []

            def outp_units(ctxN_pair, q0_prev, sb, fine=False):
                # shared o_sb tile across the sb's two ec half-units; `fine`
                # (last chunk) DMAs per-ec and alternates the bias add onto
                # gpsimd so the closing tail isn't serialized on one engine.
                cell = {}

                def emit_ec(ec):
                    def emit():
                        if "o_sb" not in cell:
                            cell["o_sb"] = outp.tile(
                                [128, E], BF16, tag="osb", name="o_sb"
                            )
                        o_sb = cell["o_sb"]
                        rs = slice(q0_prev + sb * 128, q0_prev + (sb + 1) * 128)
                        es_ = slice(ec * 512, (ec + 1) * 512)
                        o_ps = op.tile([128, 512], F32, tag="ctx", name="o_ps")
                        nc.tensor.matmul(
                            o_ps[:],
                            ctxN_pair[0][:, ls_of(sb)],
                            wovT[0][:, es_],
                            start=True,
                            stop=False,
                        )
                        nc.tensor.matmul(
                            o_ps[:],
                            ctxN_pair[1][:, ls_of(sb)],
                            wovT[1][:, es_],
                            start=False,
                            stop=True,
                        )
                        nc.vector.tensor_add(o_sb[:, es_], o_ps[:], bo4_bc[:, es_])
                        if fine:
                            nc.sync.dma_start(out_d[rs, es_], o_sb[:, es_])
                        elif ec == E // 512 - 1:
                            nc.sync.dma_start(out_d[rs, :], o_sb[:])

                    return emit

                return [emit_ec(ec) for ec in range(E // 512)]

            def ls_of(sb):
                return slice(sb * 128, (sb + 1) * 128)

            def drain_one():
                if pending:
                    pending.pop(0)()

            # ---- flat item stream: (chunk, pair, block) ----
            items = []
            for oo, ci in enumerate(chunk_order):
                blks = schedule[ci]
                nb = len(blks)
                n_items = 2 * nb
                for p in range(2):
                    for bi, (blk, mode, aux) in enumerate(blks):
                        idx = p * nb + bi
                        items.append(
                            dict(
                                ci=ci,
                                p=p,
                                blk=blk,
                                mode=mode,
                                aux=aux,
                                first=(bi == 0),
                                last=(bi == len(blks) - 1),
                                hoist_kA=(
                                    chunk_order[oo + 1]
                                    if p == 0 and bi == nb // 2 and oo + 1 < N_CHUNK
                                    else None
                                ),
                                fine=(oo == N_CHUNK - 1),
                                # pace the deferred outP drains evenly over the
                                # chunk's items: how many should have drained
                                # by the end of this item
                                drain_mark=((idx + 1) * 8) // n_items,
                            )
                        )

            state = {"ctx_ps": None, "ctxN": {}, "drained": 0}

            def emit_scores(it):
                ci, p, blk, mode = it["ci"], it["p"], it["blk"], it["mode"]
                q0 = ci * CHUNK
                c0 = max(0, blk * 128 - q0) if mode == "causal" else 0
                it["c0"] = c0
                s_ps = sp.tile([128, 2 * CHUNK], F32, tag="scores", name="s_ps")
                es = es_pool.tile([128, 2 * CHUNK], BF16, tag="es", name="es")
                it["es"] = es
                ks = slice(blk * 128, (blk + 1) * 128)
                for hh in range(2):
                    o = hh * D
                    nc.tensor.matmul(
                        s_ps[:, hh * CHUNK + c0 : (hh + 1) * CHUNK],
                        kAT[p][o : o + D, ks],
                        qT[p][o : o + D, q0 + c0 : q0 + CHUNK],
                        start=True,
                        stop=True,
                    )
                if c0 == 0:
                    nc.scalar.activation(es[:], s_ps[:], Exp)
                else:
                    for hh in range(2):
                        hs0 = hh * CHUNK
                        nc.scalar.activation(
                            es[:, hs0 + c0 : hs0 + CHUNK],
                            s_ps[:, hs0 + c0 : hs0 + CHUNK],
                            Exp,
                        )
                # masking is SBUF-only: split the two heads across DVE and
                # gpsimd so their serial latency halves and neither engine
                # carries the whole load
                if mode == "causal":
                    for hh in range(2):
                        hs0 = hh * CHUNK
                        eng = nc.vector if hh == 0 else nc.gpsimd
                        eng.tensor_mul(
                            es[:, hs0 + c0 : hs0 + c0 + 128],
                            es[:, hs0 + c0 : hs0 + c0 + 128],
                            tri_sb[:],
                        )
                elif mode == "tile":
                    for hh in range(2):
                        hs0 = hh * CHUNK
                        eng = nc.vector if hh == 0 else nc.gpsimd
                        eng.tensor_mul(
                            es[:, hs0 : hs0 + CHUNK],
                            es[:, hs0 : hs0 + CHUNK],
                            mtiles[it["aux"]][:],
                        )

            def emit_ctx(it):
                ci, p, blk, c0 = it["ci"], it["p"], it["blk"], it["c0"]
                q0 = ci * CHUNK
                es = it["es"]
                if it["first"]:
                    state["ctx_ps"] = [
                        cp.tile([D + 1, CHUNK], F32, tag="ctx", name=f"ctx{hh}")
                        for hh in range(2)
                    ]
                ctx_ps = state["ctx_ps"]
                for hh in range(2):
                    h = 2 * p + hh
                    nc.tensor.matmul(
                        ctx_ps[hh][:, c0:],
                        va[h][:, blk * (D + 1) : (blk + 1) * (D + 1)],
                        es[:, hh * CHUNK + c0 : (hh + 1) * CHUNK],
                        start=it["first"],
                        stop=it["last"],
                    )
                while state["drained"] < it["drain_mark"] and pending:
                    drain_one()
                    state["drained"] += 1
                if it["last"]:
                    # normalize: ctxN = ctxU * (1/r); r_inv broadcast to 64
                    # partitions via a K=1 PE matmul (ones outer r_inv)
                    ctxN_p = ctxn.tile([128, CHUNK], F32R, tag="ctxN")
                    r_invs = []
                    for hh in range(2):
                        r_inv = rr.tile([1, CHUNK], F32, tag="rinv")
                        nc.vector.reciprocal(r_inv[:], ctx_ps[hh][D : D + 1, :])
                        r_invs.append(r_inv)
                    for hh in range(2):
                        o = hh * D
                        if K_BCAST == "pe":
                            r_bc = op.tile([D, CHUNK], F32, tag="ctx", name="r_bc")
                            nc.tensor.matmul(
                                r_bc[:],
                                ones_sb[:],
                                r_invs[hh][:].bitcast(F32R),
                                start=True,
                                stop=True,
                            )
                        else:
                            r_bc = rr.tile([D, CHUNK], F32, tag="rbc")
                            nc.gpsimd.partition_broadcast(r_bc[:], r_invs[hh][:])
                        nc.vector.tensor_mul(
                            ctxN_p[o : o + D, :], ctx_ps[hh][0:D, :], r_bc[:]
                        )
                    state["ctxN"][p] = ctxN_p
                    if p == 1:
                        pair = (state["ctxN"][0], state["ctxN"][1])
                        for sb in range(CHUNK // 128):
                            pending.extend(
                                outp_units(pair, q0, sb, fine=it["fine"])
                            )
                        state["drained"] = 0

            emit_kA(chunk_order[0])
            prev = None
            for it in items:
                emit_scores(it)
                if prev is not None:
                    emit_ctx(prev)
                prev = it
                if it["hoist_kA"] is not None:
                    emit_kA(it["hoist_kA"])
            emit_ctx(prev)
            while pending:
                drain_one()

        if hw_loop:
            _emit_prep()
            with tc.For_i(0, hw_loop) as _i:
                _emit_body(False)
        else:
            for _rep in range(repeat):
                _emit_body(_rep == 0)

    nc.compile()
    return nc


def prepare(key, query, value, mask, Wq, Wk, Wv, Wo, bo, build=True):
    """Host-side sharding/layout prep. Returns (nc, in_maps, gather)."""
    key = np.asarray(key, dtype=np.float32)
    query = np.asarray(query, dtype=np.float32)
    value = np.asarray(value, dtype=np.float32)
    Wq = np.asarray(Wq, dtype=np.float32)
    Wk = np.asarray(Wk, dtype=np.float32)
    Wv = np.asarray(Wv, dtype=np.float32)
    Wo = np.asarray(Wo, dtype=np.float32)
    bo = np.asarray(bo, dtype=np.float32)

    schedule, mtiles = _analyze_mask(mask)
    nc = build_nc(schedule, len(mtiles)) if build else None

    woT_all = np.ascontiguousarray(Wo.T.reshape(H, D, E))  # per head: Wo[:, cols_h].T
    bo4 = (bo / 4.0).reshape(1, E)
    mt = np.stack(mtiles).astype(BF16_NP) if mtiles else None
    tri = (
        (np.arange(128)[None, :] >= np.arange(128)[:, None])
        .astype(BF16_NP)
        .reshape(128, 128)
    )

    in_maps = []
    for c in range(N_CORES):
        b = c // 4
        h0 = 4 * (c % 4)
        hs = slice(h0, h0 + 4)
        q = query[b].reshape(S, H, D)[:, hs, :]  # [S, 4, D]
        k = key[b].reshape(S, H, D)[:, hs, :]
        v = value[b].reshape(S, H, D)[:, hs, :]
        # pair-stacked transposed layouts [2, 128, S]
        qT = np.ascontiguousarray(q.transpose(1, 2, 0).reshape(2, 2 * D, S))
        kT = np.ascontiguousarray(k.transpose(1, 2, 0).reshape(2, 2 * D, S))
        va = np.ones((4, S, D + 1), dtype=np.float32)
        va[:, :, :D] = v.transpose(1, 0, 2)
        # partition-major: [4, S, D+1] -> [4, 128, N_BLK*(D+1)]
        va = (
            va.reshape(4, N_BLK, 128, D + 1)
            .transpose(0, 2, 1, 3)
            .reshape(4, 128, N_BLK * (D + 1))
        )
        m = {
            "qT": qT.astype(BF16_NP),
            "kT": kT.astype(BF16_NP),
            "va": np.ascontiguousarray(va).astype(BF16_NP),
            "tri": tri,
            "wq": Wq,
            "wk": Wk,
            "wv": Wv,
            "woT": woT_all[h0 : h0 + 4],
            "bo4": bo4,
        }
        if mt is not None:
            m["mtiles"] = mt
        in_maps.append(m)

    def gather(results):
        out = np.empty((B, S, E), dtype=np.float32)
        for b in range(B):
            acc = results[4 * b]["out"].astype(np.float32).copy()
            for c in range(4 * b + 1, 4 * b + 4):
                acc += results[c]["out"]
            out[b] = acc
        return out

    return nc, in_maps, gather


def kernel(key, query, value, mask, Wq, Wk, Wv, Wo, bo):
    nc, in_maps, gather = prepare(key, query, value, mask, Wq, Wk, Wv, Wo, bo)
    res = run_bass_kernel_spmd(nc, in_maps, core_ids=list(range(N_CORES)))
    return gather(res.results)


# revision 41
# speedup vs baseline: 1.4209x; 1.4209x over previous
"""MultiHeadAttention Trainium2 kernel (v3).

B=2, S=2048, E=1024, H=16, D=64. 8 NeuronCores.

Sharding: B*H = 32 (batch, head) pairs -> 4 heads per core (core c handles
batch c//4, heads 4*(c%4)..4*(c%4)+3). Out-projection is column-sharded by
head (Wo folded with Wv); partial [S, E] outputs are summed on host (the
"all-reduce"), each core adding bo/4 so the sum carries the bias exactly once.

Math (per head h):
  S_scores = (q @ Wq.T) @ (k @ Wk.T).T / sqrt(D)  ==  q @ (A/8) @ k.T,
    A = Wq.T @ Wk  (so q needs no projection on device)
  P = softmax(mask(S_scores))  (unnormalized exp + ones-column trick)
  ctx = P @ v  (raw v; Wv folded into Wo)
  out_h = ctx @ (Wo[:, cols_h] @ Wv).T

v3 schedule: one work item = one (sq-chunk, head-pair, sk-block). Its score
tile [128, 2*CHUNK] holds BOTH heads of the pair side by side, so a single
Activation instruction exps both heads, and one item consumes one PSUM
buffer (bufs=2 gives one item of lookahead). Items stream flat across
pairs and chunks with scores emitted one item ahead of the exp->ctx
consumers; kA for chunk ci+1 is hoisted into chunk ci; the chunk's output
projection is deferred and drained between the next chunk's items to fill
PE gaps. The q/k score path runs in bf16 (fp32r has an N>=256 fast-path
constraint; bf16 allows exact causal column starts), the v/es ctx path in
bf16, and the out-projection in fp32r. Causal masking is a [128,128]
triangle bf16 multiply on DVE; exp is the only Activation-engine op.
"""

import sys

if "/opt/trn_rl_repo" not in sys.path:
    sys.path.insert(0, "/opt/trn_rl_repo")

import os

import numpy as np

import concourse.bass as bass
import concourse.tile as tile
from concourse import bacc, mybir
from concourse.bass_utils import run_bass_kernel_spmd

K_KA_POOL = os.environ.get("K_KA_POOL", "sp")  # sp | op
K_BCAST = os.environ.get("K_BCAST", "pool")  # pe | pool

B, S, E, H = 2, 2048, 1024, 16
D = E // H  # 64
N_CORES = 8
HEADS_PER_CORE = H * B // N_CORES  # 4
N_CHUNK = 4  # sq chunks of 512
CHUNK = S // N_CHUNK  # 512
N_BLK = S // 128  # 16 sk blocks of 128
F32 = mybir.dt.float32
F32R = mybir.dt.float32r
BF16 = mybir.dt.bfloat16
BF16_NP = mybir.dt.np(mybir.dt.bfloat16)


def _analyze_mask(mask):
    """Classify each (sq-chunk, sk-block) region of the shared mask.

    Returns (schedule, tiles): schedule[ci] is a list of (blk, mode, aux)
    with mode in {"plain", "causal", "tile"}; tiles is the list of distinct
    float32 [128, CHUNK] (sk, sq) multiplicative mask tiles for "tile" mode.
    """
    m = np.asarray(mask).reshape(S, S) != 0
    schedule = []
    tiles = []
    tile_index = {}
    for ci in range(N_CHUNK):
        q0 = ci * CHUNK
        blks = []
        for k in range(N_BLK):
            k0 = k * 128
            mb = m[q0 : q0 + CHUNK, k0 : k0 + 128]  # [sq, sk]
            if not mb.any():
                continue
            if mb.all():
                blks.append((k, "plain", None))
                continue
            causal = (
                np.arange(q0, q0 + CHUNK)[:, None] >= np.arange(k0, k0 + 128)[None, :]
            )
            if np.array_equal(mb, causal):
                blks.append((k, "causal", None))
            else:
                t = np.ascontiguousarray(mb.T.astype(np.float32))  # [sk, sq]
                key = t.tobytes()
                if key not in tile_index:
                    tile_index[key] = len(tiles)
                    tiles.append(t)
                blks.append((k, "tile", tile_index[key]))
        schedule.append(blks)
    return schedule, tiles


def build_nc(schedule, n_mask_tiles, repeat=1, hw_loop=0):
    """Build the SPMD Bass program (identical for all 8 cores)."""
    nc = bacc.Bacc(
        "TRN2", target_bir_lowering=False, debug=False, num_devices=N_CORES
    )

    qT_d = nc.dram_tensor("qT", [2, 128, S], BF16, kind="ExternalInput").ap()
    kT_d = nc.dram_tensor("kT", [2, 128, S], BF16, kind="ExternalInput").ap()
    va_d = nc.dram_tensor(
        "va", [4, 128, N_BLK * (D + 1)], BF16, kind="ExternalInput"
    ).ap()
    tri_d = nc.dram_tensor("tri", [128, 128], BF16, kind="ExternalInput").ap()
    wq_d = nc.dram_tensor("wq", [D, D], F32, kind="ExternalInput").ap()
    wk_d = nc.dram_tensor("wk", [D, D], F32, kind="ExternalInput").ap()
    wv_d = nc.dram_tensor("wv", [D, D], F32, kind="ExternalInput").ap()
    woT_d = nc.dram_tensor("woT", [4, D, E], F32, kind="ExternalInput").ap()
    bo4_d = nc.dram_tensor("bo4", [1, E], F32, kind="ExternalInput").ap()
    if n_mask_tiles:
        mt_d = nc.dram_tensor(
            "mtiles", [n_mask_tiles, 128, CHUNK], BF16, kind="ExternalInput"
        ).ap()
    out_d = nc.dram_tensor("out", [S, E], BF16, kind="ExternalOutput").ap()

    Exp = mybir.ActivationFunctionType.Exp

    from contextlib import ExitStack

    with tile.TileContext(nc) as tc, ExitStack() as ctx:
        const = ctx.enter_context(tc.tile_pool(name="const", bufs=1))
        qk = ctx.enter_context(tc.tile_pool(name="qk", bufs=1))
        va_pool = ctx.enter_context(tc.tile_pool(name="vap", bufs=1))
        es_pool = ctx.enter_context(tc.tile_pool(name="es", bufs=8))
        ctxn = ctx.enter_context(tc.tile_pool(name="ctxn", bufs=6))
        rr = ctx.enter_context(tc.tile_pool(name="rr", bufs=4))
        outp = ctx.enter_context(tc.tile_pool(name="outp", bufs=3))
        sp = ctx.enter_context(tc.tile_pool(name="sp", bufs=2, space="PSUM"))
        cp = ctx.enter_context(tc.tile_pool(name="cp", bufs=2, space="PSUM"))
        op = ctx.enter_context(tc.tile_pool(name="op", bufs=2, space="PSUM"))

        # ---- constants / weight prep ----
        wq_sb = const.tile([D, D], F32, tag="wq")
        # Wk loaded twice side by side: the A.T matmul then yields A.T
        # replicated on partitions 0-63 and 64-127 in one shot
        wk2_sb = const.tile([D, 2 * D], F32, tag="wk2")
        wv_sb = const.tile([D, D], F32R, tag="wv")
        nc.sync.dma_start(wq_sb[:], wq_d[:])
        nc.sync.dma_start(wk2_sb[:, 0:D], wk_d[:])
        nc.sync.dma_start(wk2_sb[:, D : 2 * D], wk_d[:])
        nc.sync.dma_start(wv_sb[:], wv_d[:].bitcast(F32R))

        # A.T/8 = (Wk.T @ Wq)/8  [d', d], replicated over both partition halves
        at_ps = sp.tile([128, D], F32, tag="scores")
        nc.tensor.matmul(at_ps[:], wk2_sb[:], wq_sb[:], start=True, stop=True)
        at_sb = const.tile([128, D], BF16, tag="at")
        nc.vector.tensor_scalar_mul(at_sb[:], at_ps[:], 1.0 / np.sqrt(float(D)))
        # ones row for the K=1 broadcast matmul (r_inv -> r_bc in PSUM)
        ones_sb = None
        if K_BCAST == "pe":
            ones_sb = const.tile([1, D], F32R, tag="ones")
            nc.vector.memset(ones_sb[:], 1.0)

        wovT, mtiles = [], []
        bo4_bc = None
        tri_sb = const.tile([128, 128], BF16, tag="tri")

        def _emit_prep():
            nonlocal bo4_bc
            nc.sync.dma_start(tri_sb[:], tri_d[:])
            # dummy exp so the Exp activation table loads during prep, not
            # inside the hw loop body
            warm = const.tile([1, 1], F32, tag="warm")
            nc.scalar.activation(warm[:], wq_sb[0:1, 0:1], Exp)
            for p in range(2):
                wovT_p = const.tile([128, E], F32R, tag=f"wovT{p}", name=f"wovT{p}")
                wovT.append(wovT_p)
            for h in range(4):
                woT_sb = const.tile([D, E], F32R, tag="woT_ld")
                nc.sync.dma_start(woT_sb[:], woT_d[h].bitcast(F32R))
                p, o = h // 2, (h % 2) * D
                for ec in range(E // 512):
                    wo_ps = op.tile([D, 512], F32, tag="ctx")
                    nc.tensor.matmul(
                        wo_ps[:],
                        wv_sb[:],
                        woT_sb[:, ec * 512 : (ec + 1) * 512],
                        start=True,
                        stop=True,
                    )
                    nc.vector.tensor_copy(
                        wovT[p][o : o + D, ec * 512 : (ec + 1) * 512], wo_ps[:]
                    )
            bo4_sb = const.tile([1, E], F32, tag="bo4")
            nc.sync.dma_start(bo4_sb[:], bo4_d[:])
            bo4_bc = const.tile([128, E], F32, tag="bo4bc")
            nc.gpsimd.partition_broadcast(bo4_bc[:], bo4_sb[:])
            for i in range(n_mask_tiles):
                t = const.tile([128, CHUNK], BF16, tag=f"mt{i}", name=f"mt{i}")
                nc.sync.dma_start(t[:], mt_d[i])
                mtiles.append(t)

        def _emit_body(_first):
            # ---- input loads, ci-major so the pipeline can start early ----
            qT = []
            kAT = []
            va = []
            k_sb_l = []
            for p in range(2):
                qT.append(qk.tile([128, S], BF16, tag=f"qT{p}", name=f"qT{p}"))
                k_sb_l.append(qk.tile([128, S], BF16, tag=f"kT{p}", name=f"kT{p}"))
                kAT.append(qk.tile([128, S], BF16, tag=f"kAT{p}", name=f"kAT{p}"))
            for h in range(4):
                va.append(
                    va_pool.tile(
                        [128, N_BLK * (D + 1)], BF16, tag=f"va{h}", name=f"va{h}"
                    )
                )
            # natural order: the JIT kA only adds each chunk's new sk columns,
            # so chunks must be visited in ascending sk coverage
            chunk_order = list(range(N_CHUNK))
            for oi, ci in enumerate(chunk_order):
                cs = slice(ci * CHUNK, (ci + 1) * CHUNK)
                for p in range(2):
                    nc.sync.dma_start(k_sb_l[p][:, cs], kT_d[p, :, cs])
                    nc.sync.dma_start(qT[p][:, cs], qT_d[p, :, cs])
                if oi < 2:
                    for hh in range(2):
                        h = 2 * oi + hh
                        nc.gpsimd.dma_start(va[h][:], va_d[h])

            if _first and not hw_loop:
                _emit_prep()

            def emit_kA(ci):
                # all 4 (pair, head) kA matmuls land in ONE score-pool tile
                # (disjoint partition/column quadrants), so the hoisted kA
                # costs a single PSUM slot-wait and two gpsimd copies.
                cs_k = slice(ci * CHUNK, (ci + 1) * CHUNK)
                ka_ps = sp.tile([128, 2 * CHUNK], F32, tag="scores", name="ka_ps")
                for p_ in range(2):
                    for hh in range(2):
                        o = hh * D
                        nc.tensor.matmul(
                            ka_ps[o : o + D, p_ * CHUNK : (p_ + 1) * CHUNK],
                            at_sb[o : o + D, :],
                            k_sb_l[p_][o : o + D, cs_k],
                            start=True,
                            stop=True,
                        )
                # gpsimd cannot read PSUM; these copies must ride DVE
                for p_ in range(2):
                    nc.vector.tensor_copy(
                        kAT[p_][:, cs_k], ka_ps[:, p_ * CHUNK : (p_ + 1) * CHUNK]
                    )

            # ---- deferred output-projection half-units (one per (sb, ec)) ----
            pending = []

            def outp_units(ctxN_pair, q0_prev, sb, fine=False):
                # shared o_sb tile across the sb's two ec half-units; `fine`
                # (last chunk) DMAs per-ec and alternates the bias add onto
                # gpsimd so the closing tail isn't serialized on one engine.
                cell = {}

                def emit_ec(ec):
                    def emit():
                        if "o_sb" not in cell:
                            cell["o_sb"] = outp.tile(
                                [128, E], BF16, tag="osb", name="o_sb"
                            )
                        o_sb = cell["o_sb"]
                        rs = slice(q0_prev + sb * 128, q0_prev + (sb + 1) * 128)
                        es_ = slice(ec * 512, (ec + 1) * 512)
                        o_ps = op.tile([128, 512], F32, tag="ctx", name="o_ps")
                        ls = slice(sb * 128, (sb + 1) * 128)
                        nc.tensor.matmul(
                            o_ps[:],
                            ctxN_pair[0][:, ls],
                            wovT[0][:, es_],
                            start=True,
                            stop=False,
                        )
                        nc.tensor.matmul(
                            o_ps[:],
                            ctxN_pair[1][:, ls],
                            wovT[1][:, es_],
                            start=False,
                            stop=True,
                        )
                        nc.vector.tensor_add(o_sb[:, es_], o_ps[:], bo4_bc[:, es_])
                        if fine:
                            nc.sync.dma_start(out_d[rs, es_], o_sb[:, es_])
                        elif ec == E // 512 - 1:
                            nc.sync.dma_start(out_d[rs, :], o_sb[:])

                    return emit

                return [emit_ec(ec) for ec in range(E // 512)]

            def drain_one():
                if pending:
                    pending.pop(0)()

            # ---- flat item stream: (chunk, pair, block) ----
            items = []
            for oo, ci in enumerate(chunk_order):
                blks = schedule[ci]
                nb = len(blks)
                n_items = 2 * nb
                for p in range(2):
                    for bi, (blk, mode, aux) in enumerate(blks):
                        idx = p * nb + bi
                        items.append(
                            dict(
                                ci=ci,
                                p=p,
                                blk=blk,
                                mode=mode,
                                aux=aux,
                                first=(bi == 0),
                                last=(bi == len(blks) - 1),
                                hoist_kA=(
                                    chunk_order[oo + 1]
                                    if p == 0 and bi == nb // 2 and oo + 1 < N_CHUNK
                                    else None
                                ),
                                fine=(oo == N_CHUNK - 1),
                                # pace the deferred outP drains evenly over the
                                # chunk's items: how many should have drained
                                # by the end of this item
                                drain_mark=((idx + 1) * 8) // n_items,
                            )
                        )

            state = {"ctx_ps": None, "ctxN": {}, "drained": 0}

            def emit_scores(it):
                ci, p, blk, mode = it["ci"], it["p"], it["blk"], it["mode"]
                q0 = ci * CHUNK
                c0 = max(0, blk * 128 - q0) if mode == "causal" else 0
                it["c0"] = c0
                s_ps = sp.tile([128, 2 * CHUNK], F32, tag="scores", name="s_ps")
                es = es_pool.tile([128, 2 * CHUNK], BF16, tag="es", name="es")
                it["es"] = es
                # M<=64 matmuls double-pump on TRN2 (~2x faster per moving
                # row, near-zero weight-load cost): emit each head's score
                # block as two M=64 matmuls over disjoint sk partition halves
                for hh in range(2):
                    o = hh * D
                    for sk in range(2):
                        ks = slice(blk * 128 + sk * 64, blk * 128 + (sk + 1) * 64)
                        nc.tensor.matmul(
                            s_ps[64 * sk : 64 * sk + 64, hh * CHUNK + c0 : (hh + 1) * CHUNK],
                            kAT[p][o : o + D, ks],
                            qT[p][o : o + D, q0 + c0 : q0 + CHUNK],
                            start=True,
                            stop=True,
                        )
                if c0 == 0:
                    nc.scalar.activation(es[:], s_ps[:], Exp)
                else:
                    for hh in range(2):
                        hs0 = hh * CHUNK
                        nc.scalar.activation(
                            es[:, hs0 + c0 : hs0 + CHUNK],
                            s_ps[:, hs0 + c0 : hs0 + CHUNK],
                            Exp,
                        )
                # masking: bf16 SBUF multiplies are ~14ns on DVE (measured);
                # gpsimd software ops cost ~40x that, so everything stays DVE
                if mode == "causal":
                    for hh in range(2):
                        hs0 = hh * CHUNK
                        nc.vector.tensor_mul(
                            es[:, hs0 + c0 : hs0 + c0 + 128],
                            es[:, hs0 + c0 : hs0 + c0 + 128],
                            tri_sb[:],
                        )
                elif mode == "tile":
                    for hh in range(2):
                        hs0 = hh * CHUNK
                        nc.vector.tensor_mul(
                            es[:, hs0 : hs0 + CHUNK],
                            es[:, hs0 : hs0 + CHUNK],
                            mtiles[it["aux"]][:],
                        )

            def emit_ctx(it):
                ci, p, blk, c0 = it["ci"], it["p"], it["blk"], it["c0"]
                q0 = ci * CHUNK
                es = it["es"]
                if it["first"]:
                    state["ctx_ps"] = [
                        cp.tile([D + 1, CHUNK], F32, tag="ctx", name=f"ctx{hh}")
                        for hh in range(2)
                    ]
                ctx_ps = state["ctx_ps"]
                for hh in range(2):
                    h = 2 * p + hh
                    nc.tensor.matmul(
                        ctx_ps[hh][:, c0:],
                        va[h][:, blk * (D + 1) : (blk + 1) * (D + 1)],
                        es[:, hh * CHUNK + c0 : (hh + 1) * CHUNK],
                        start=it["first"],
                        stop=it["last"],
                    )
                while state["drained"] < it["drain_mark"] and pending:
                    drain_one()
                    state["drained"] += 1
                if it["last"]:
                    # normalize: ctxN = ctxU * (1/r); r_inv broadcast to 64
                    # partitions via a K=1 PE matmul (ones outer r_inv)
                    ctxN_p = ctxn.tile([128, CHUNK], F32R, tag="ctxN")
                    r_invs = []
                    for hh in range(2):
                        r_inv = rr.tile([1, CHUNK], F32, tag="rinv")
                        nc.vector.reciprocal(r_inv[:], ctx_ps[hh][D : D + 1, :])
                        r_invs.append(r_inv)
                    for hh in range(2):
                        o = hh * D
                        if K_BCAST == "pe":
                            r_bc = op.tile([D, CHUNK], F32, tag="ctx", name="r_bc")
                            nc.tensor.matmul(
                                r_bc[:],
                                ones_sb[:],
                                r_invs[hh][:].bitcast(F32R),
                                start=True,
                                stop=True,
                            )
                        else:
                            r_bc = rr.tile([D, CHUNK], F32, tag="rbc")
                            nc.gpsimd.partition_broadcast(r_bc[:], r_invs[hh][:])
                        nc.vector.tensor_mul(
                            ctxN_p[o : o + D, :], ctx_ps[hh][0:D, :], r_bc[:]
                        )
                    state["ctxN"][p] = ctxN_p
                    if p == 1:
                        pair = (state["ctxN"][0], state["ctxN"][1])
                        for sb in range(CHUNK // 128):
                            pending.extend(
                                outp_units(pair, q0, sb, fine=it["fine"])
                            )
                        state["drained"] = 0

            emit_kA(chunk_order[0])
            prev = None
            for it in items:
                emit_scores(it)
                if prev is not None:
                    emit_ctx(prev)
                prev = it
                if it["hoist_kA"] is not None:
                    emit_kA(it["hoist_kA"])
            emit_ctx(prev)
            while pending:
                drain_one()

        if hw_loop:
            _emit_prep()
            with tc.For_i(0, hw_loop) as _i:
                _emit_body(False)
        else:
            for _rep in range(repeat):
                _emit_body(_rep == 0)

    nc.compile()
    return nc


def prepare(key, query, value, mask, Wq, Wk, Wv, Wo, bo, build=True):
    """Host-side sharding/layout prep. Returns (nc, in_maps, gather)."""
    key = np.asarray(key, dtype=np.float32)
    query = np.asarray(query, dtype=np.float32)
    value = np.asarray(value, dtype=np.float32)
    Wq = np.asarray(Wq, dtype=np.float32)
    Wk = np.asarray(Wk, dtype=np.float32)
    Wv = np.asarray(Wv, dtype=np.float32)
    Wo = np.asarray(Wo, dtype=np.float32)
    bo = np.asarray(bo, dtype=np.float32)

    schedule, mtiles = _analyze_mask(mask)
    nc = build_nc(schedule, len(mtiles)) if build else None

    woT_all = np.ascontiguousarray(Wo.T.reshape(H, D, E))  # per head: Wo[:, cols_h].T
    bo4 = (bo / 4.0).reshape(1, E)
    mt = np.stack(mtiles).astype(BF16_NP) if mtiles else None
    tri = (
        (np.arange(128)[None, :] >= np.arange(128)[:, None])
        .astype(BF16_NP)
        .reshape(128, 128)
    )

    in_maps = []
    for c in range(N_CORES):
        b = c // 4
        h0 = 4 * (c % 4)
        hs = slice(h0, h0 + 4)
        q = query[b].reshape(S, H, D)[:, hs, :]  # [S, 4, D]
        k = key[b].reshape(S, H, D)[:, hs, :]
        v = value[b].reshape(S, H, D)[:, hs, :]
        # pair-stacked transposed layouts [2, 128, S]
        qT = np.ascontiguousarray(q.transpose(1, 2, 0).reshape(2, 2 * D, S))
        kT = np.ascontiguousarray(k.transpose(1, 2, 0).reshape(2, 2 * D, S))
        va = np.ones((4, S, D + 1), dtype=np.float32)
        va[:, :, :D] = v.transpose(1, 0, 2)
        # partition-major: [4, S, D+1] -> [4, 128, N_BLK*(D+1)]
        va = (
            va.reshape(4, N_BLK, 128, D + 1)
            .transpose(0, 2, 1, 3)
            .reshape(4, 128, N_BLK * (D + 1))
        )
        m = {
            "qT": qT.astype(BF16_NP),
            "kT": kT.astype(BF16_NP),
            "va": np.ascontiguousarray(va).astype(BF16_NP),
            "tri": tri,
            "wq": Wq,
            "wk": Wk,
            "wv": Wv,
            "woT": woT_all[h0 : h0 + 4],
            "bo4": bo4,
        }
        if mt is not None:
            m["mtiles"] = mt
        in_maps.append(m)

    def gather(results):
        out = np.empty((B, S, E), dtype=np.float32)
        for b in range(B):
            acc = results[4 * b]["out"].astype(np.float32).copy()
            for c in range(4 * b + 1, 4 * b + 4):
                acc += results[c]["out"]
            out[b] = acc
        return out

    return nc, in_maps, gather


def kernel(key, query, value, mask, Wq, Wk, Wv, Wo, bo):
    nc, in_maps, gather = prepare(key, query, value, mask, Wq, Wk, Wv, Wo, bo)
    res = run_bass_kernel_spmd(nc, in_maps, core_ids=list(range(N_CORES)))
    return gather(res.results)


# revision 43
# speedup vs baseline: 1.4711x; 1.0354x over previous
"""MultiHeadAttention Trainium2 kernel (v3).

B=2, S=2048, E=1024, H=16, D=64. 8 NeuronCores.

Sharding: B*H = 32 (batch, head) pairs -> 4 heads per core (core c handles
batch c//4, heads 4*(c%4)..4*(c%4)+3). Out-projection is column-sharded by
head (Wo folded with Wv); partial [S, E] outputs are summed on host (the
"all-reduce"), each core adding bo/4 so the sum carries the bias exactly once.

Math (per head h):
  S_scores = (q @ Wq.T) @ (k @ Wk.T).T / sqrt(D)  ==  q @ (A/8) @ k.T,
    A = Wq.T @ Wk  (so q needs no projection on device)
  P = softmax(mask(S_scores))  (unnormalized exp + ones-column trick)
  ctx = P @ v  (raw v; Wv folded into Wo)
  out_h = ctx @ (Wo[:, cols_h] @ Wv).T

v3 schedule: one work item = one (sq-chunk, head-pair, sk-block). Its score
tile [128, 2*CHUNK] holds BOTH heads of the pair side by side, so a single
Activation instruction exps both heads, and one item consumes one PSUM
buffer (bufs=2 gives one item of lookahead). Items stream flat across
pairs and chunks with scores emitted one item ahead of the exp->ctx
consumers; kA for chunk ci+1 is hoisted into chunk ci; the chunk's output
projection is deferred and drained between the next chunk's items to fill
PE gaps. The q/k score path runs in bf16 (fp32r has an N>=256 fast-path
constraint; bf16 allows exact causal column starts), the v/es ctx path in
bf16, and the out-projection in fp32r. Causal masking is a [128,128]
triangle bf16 multiply on DVE; exp is the only Activation-engine op.
"""

import sys

if "/opt/trn_rl_repo" not in sys.path:
    sys.path.insert(0, "/opt/trn_rl_repo")

import os

import numpy as np

import concourse.bass as bass
import concourse.tile as tile
from concourse import bacc, mybir
from concourse.bass_utils import run_bass_kernel_spmd

K_KA_POOL = os.environ.get("K_KA_POOL", "sp")  # sp | op
K_BCAST = os.environ.get("K_BCAST", "pool")  # pe | pool

B, S, E, H = 2, 2048, 1024, 16
D = E // H  # 64
N_CORES = 8
HEADS_PER_CORE = H * B // N_CORES  # 4
N_CHUNK = 4  # sq chunks of 512
CHUNK = S // N_CHUNK  # 512
N_BLK = S // 128  # 16 sk blocks of 128
F32 = mybir.dt.float32
F32R = mybir.dt.float32r
BF16 = mybir.dt.bfloat16
BF16_NP = mybir.dt.np(mybir.dt.bfloat16)


def _analyze_mask(mask):
    """Classify each (sq-chunk, sk-block) region of the shared mask.

    Returns (schedule, tiles): schedule[ci] is a list of (blk, mode, aux)
    with mode in {"plain", "causal", "tile"}; tiles is the list of distinct
    float32 [128, CHUNK] (sk, sq) multiplicative mask tiles for "tile" mode.
    """
    m = np.asarray(mask).reshape(S, S) != 0
    schedule = []
    tiles = []
    tile_index = {}
    for ci in range(N_CHUNK):
        q0 = ci * CHUNK
        blks = []
        for k in range(N_BLK):
            k0 = k * 128
            mb = m[q0 : q0 + CHUNK, k0 : k0 + 128]  # [sq, sk]
            if not mb.any():
                continue
            if mb.all():
                blks.append((k, "plain", None))
                continue
            causal = (
                np.arange(q0, q0 + CHUNK)[:, None] >= np.arange(k0, k0 + 128)[None, :]
            )
            if np.array_equal(mb, causal):
                blks.append((k, "causal", None))
            else:
                t = np.ascontiguousarray(mb.T.astype(np.float32))  # [sk, sq]
                key = t.tobytes()
                if key not in tile_index:
                    tile_index[key] = len(tiles)
                    tiles.append(t)
                blks.append((k, "tile", tile_index[key]))
        schedule.append(blks)
    return schedule, tiles


def build_nc(schedule, n_mask_tiles, repeat=1, hw_loop=0):
    """Build the SPMD Bass program (identical for all 8 cores)."""
    nc = bacc.Bacc(
        "TRN2", target_bir_lowering=False, debug=False, num_devices=N_CORES
    )

    qT_d = nc.dram_tensor("qT", [2, 128, S], BF16, kind="ExternalInput").ap()
    kT_d = nc.dram_tensor("kT", [2, 128, S], BF16, kind="ExternalInput").ap()
    va_d = nc.dram_tensor(
        "va", [4, 128, N_BLK * (D + 1)], BF16, kind="ExternalInput"
    ).ap()
    tri_d = nc.dram_tensor("tri", [128, 128], BF16, kind="ExternalInput").ap()
    wq_d = nc.dram_tensor("wq", [D, D], F32, kind="ExternalInput").ap()
    wk_d = nc.dram_tensor("wk", [D, D], F32, kind="ExternalInput").ap()
    wv_d = nc.dram_tensor("wv", [D, D], F32, kind="ExternalInput").ap()
    woT_d = nc.dram_tensor("woT", [4, D, E], F32, kind="ExternalInput").ap()
    bo4_d = nc.dram_tensor("bo4", [1, E], F32, kind="ExternalInput").ap()
    if n_mask_tiles:
        mt_d = nc.dram_tensor(
            "mtiles", [n_mask_tiles, 128, CHUNK], BF16, kind="ExternalInput"
        ).ap()
    out_d = nc.dram_tensor("out", [S, E], BF16, kind="ExternalOutput").ap()

    Exp = mybir.ActivationFunctionType.Exp

    from contextlib import ExitStack

    with tile.TileContext(nc) as tc, ExitStack() as ctx:
        const = ctx.enter_context(tc.tile_pool(name="const", bufs=1))
        qk = ctx.enter_context(tc.tile_pool(name="qk", bufs=1))
        va_pool = ctx.enter_context(tc.tile_pool(name="vap", bufs=1))
        es_pool = ctx.enter_context(tc.tile_pool(name="es", bufs=8))
        ctxn = ctx.enter_context(tc.tile_pool(name="ctxn", bufs=6))
        rr = ctx.enter_context(tc.tile_pool(name="rr", bufs=4))
        outp = ctx.enter_context(tc.tile_pool(name="outp", bufs=3))
        sp = ctx.enter_context(tc.tile_pool(name="sp", bufs=2, space="PSUM"))
        cp = ctx.enter_context(tc.tile_pool(name="cp", bufs=2, space="PSUM"))
        op = ctx.enter_context(tc.tile_pool(name="op", bufs=2, space="PSUM"))

        # ---- constants / weight prep ----
        wq_sb = const.tile([D, D], F32, tag="wq")
        # Wk loaded twice side by side: the A.T matmul then yields A.T
        # replicated on partitions 0-63 and 64-127 in one shot
        wk2_sb = const.tile([D, 2 * D], F32, tag="wk2")
        wv_sb = const.tile([D, D], F32R, tag="wv")
        nc.sync.dma_start(wq_sb[:], wq_d[:])
        nc.sync.dma_start(wk2_sb[:, 0:D], wk_d[:])
        nc.sync.dma_start(wk2_sb[:, D : 2 * D], wk_d[:])
        nc.sync.dma_start(wv_sb[:], wv_d[:].bitcast(F32R))

        # A.T/8 = (Wk.T @ Wq)/8  [d', d], replicated over both partition halves
        at_ps = sp.tile([128, D], F32, tag="scores")
        nc.tensor.matmul(at_ps[:], wk2_sb[:], wq_sb[:], start=True, stop=True)
        at_sb = const.tile([128, D], BF16, tag="at")
        nc.vector.tensor_scalar_mul(at_sb[:], at_ps[:], 1.0 / np.sqrt(float(D)))
        # ones row for the K=1 broadcast matmul (r_inv -> r_bc in PSUM)
        ones_sb = None
        if K_BCAST == "pe":
            ones_sb = const.tile([1, D], F32R, tag="ones")
            nc.vector.memset(ones_sb[:], 1.0)

        wovT, mtiles = [], []
        bo4_bc = None
        tri_sb = const.tile([128, 128], BF16, tag="tri")

        def _emit_prep():
            nonlocal bo4_bc
            nc.sync.dma_start(tri_sb[:], tri_d[:])
            # dummy exp so the Exp activation table loads during prep, not
            # inside the hw loop body
            warm = const.tile([1, 1], F32, tag="warm")
            nc.scalar.activation(warm[:], wq_sb[0:1, 0:1], Exp)
            for p in range(2):
                wovT_p = const.tile([128, E], F32R, tag=f"wovT{p}", name=f"wovT{p}")
                wovT.append(wovT_p)
            for h in range(4):
                woT_sb = const.tile([D, E], F32R, tag="woT_ld")
                nc.sync.dma_start(woT_sb[:], woT_d[h].bitcast(F32R))
                p, o = h // 2, (h % 2) * D
                for ec in range(E // 512):
                    wo_ps = op.tile([D, 512], F32, tag="ctx")
                    nc.tensor.matmul(
                        wo_ps[:],
                        wv_sb[:],
                        woT_sb[:, ec * 512 : (ec + 1) * 512],
                        start=True,
                        stop=True,
                    )
                    nc.vector.tensor_copy(
                        wovT[p][o : o + D, ec * 512 : (ec + 1) * 512], wo_ps[:]
                    )
            bo4_sb = const.tile([1, E], F32, tag="bo4")
            nc.sync.dma_start(bo4_sb[:], bo4_d[:])
            bo4_bc = const.tile([128, E], F32, tag="bo4bc")
            nc.gpsimd.partition_broadcast(bo4_bc[:], bo4_sb[:])
            for i in range(n_mask_tiles):
                t = const.tile([128, CHUNK], BF16, tag=f"mt{i}", name=f"mt{i}")
                nc.sync.dma_start(t[:], mt_d[i])
                mtiles.append(t)

        def _emit_body(_first):
            # ---- input loads, ci-major so the pipeline can start early ----
            qT = []
            kAT = []
            va = []
            k_sb_l = []
            for p in range(2):
                qT.append(qk.tile([128, S], BF16, tag=f"qT{p}", name=f"qT{p}"))
                k_sb_l.append(qk.tile([128, S], BF16, tag=f"kT{p}", name=f"kT{p}"))
                kAT.append(qk.tile([128, S], BF16, tag=f"kAT{p}", name=f"kAT{p}"))
            for h in range(4):
                va.append(
                    va_pool.tile(
                        [128, N_BLK * (D + 1)], BF16, tag=f"va{h}", name=f"va{h}"
                    )
                )
            # natural order: the JIT kA only adds each chunk's new sk columns,
            # so chunks must be visited in ascending sk coverage
            chunk_order = list(range(N_CHUNK))
            for oi, ci in enumerate(chunk_order):
                cs = slice(ci * CHUNK, (ci + 1) * CHUNK)
                for p in range(2):
                    nc.sync.dma_start(k_sb_l[p][:, cs], kT_d[p, :, cs])
                    nc.sync.dma_start(qT[p][:, cs], qT_d[p, :, cs])
                if oi < 2:
                    for hh in range(2):
                        h = 2 * oi + hh
                        nc.gpsimd.dma_start(va[h][:], va_d[h])

            if _first and not hw_loop:
                _emit_prep()

            def emit_kA(ci):
                # all 4 (pair, head) kA matmuls land in ONE score-pool tile
                # (disjoint partition/column quadrants), so the hoisted kA
                # costs a single PSUM slot-wait and two gpsimd copies.
                cs_k = slice(ci * CHUNK, (ci + 1) * CHUNK)
                ka_ps = sp.tile([128, 2 * CHUNK], F32, tag="scores", name="ka_ps")
                for p_ in range(2):
                    for hh in range(2):
                        o = hh * D
                        nc.tensor.matmul(
                            ka_ps[o : o + D, p_ * CHUNK : (p_ + 1) * CHUNK],
                            at_sb[o : o + D, :],
                            k_sb_l[p_][o : o + D, cs_k],
                            start=True,
                            stop=True,
                        )
                # gpsimd cannot read PSUM; these copies must ride DVE
                for p_ in range(2):
                    nc.vector.tensor_copy(
                        kAT[p_][:, cs_k], ka_ps[:, p_ * CHUNK : (p_ + 1) * CHUNK]
                    )

            # ---- deferred output-projection half-units (one per (sb, ec)) ----
            pending = []

            def outp_units(ctxN_pair, q0_prev, sb, fine=False):
                # shared o_sb tile across the sb's two ec half-units; `fine`
                # (last chunk) DMAs per-ec and alternates the bias add onto
                # gpsimd so the closing tail isn't serialized on one engine.
                cell = {}

                def emit_ec(ec):
                    def emit():
                        if "o_sb" not in cell:
                            cell["o_sb"] = outp.tile(
                                [128, E], BF16, tag="osb", name="o_sb"
                            )
                        o_sb = cell["o_sb"]
                        rs = slice(q0_prev + sb * 128, q0_prev + (sb + 1) * 128)
                        es_ = slice(ec * 512, (ec + 1) * 512)
                        o_ps = op.tile([128, 512], F32, tag="ctx", name="o_ps")
                        ls = slice(sb * 128, (sb + 1) * 128)
                        nc.tensor.matmul(
                            o_ps[:],
                            ctxN_pair[0][:, ls],
                            wovT[0][:, es_],
                            start=True,
                            stop=False,
                        )
                        nc.tensor.matmul(
                            o_ps[:],
                            ctxN_pair[1][:, ls],
                            wovT[1][:, es_],
                            start=False,
                            stop=True,
                        )
                        nc.vector.tensor_add(o_sb[:, es_], o_ps[:], bo4_bc[:, es_])
                        if fine:
                            nc.sync.dma_start(out_d[rs, es_], o_sb[:, es_])
                        elif ec == E // 512 - 1:
                            nc.sync.dma_start(out_d[rs, :], o_sb[:])

                    return emit

                return [emit_ec(ec) for ec in range(E // 512)]

            def drain_one():
                if pending:
                    pending.pop(0)()

            # ---- flat item stream: (chunk, pair, block) ----
            items = []
            for oo, ci in enumerate(chunk_order):
                blks = schedule[ci]
                nb = len(blks)
                n_items = 2 * nb
                for p in range(2):
                    for bi, (blk, mode, aux) in enumerate(blks):
                        idx = p * nb + bi
                        items.append(
                            dict(
                                ci=ci,
                                p=p,
                                blk=blk,
                                mode=mode,
                                aux=aux,
                                first=(bi == 0),
                                last=(bi == len(blks) - 1),
                                hoist_kA=(
                                    chunk_order[oo + 1]
                                    if p == 0 and bi == nb // 2 and oo + 1 < N_CHUNK
                                    else None
                                ),
                                fine=(oo == N_CHUNK - 1),
                                # pace the deferred outP drains evenly over the
                                # chunk's items: how many should have drained
                                # by the end of this item
                                drain_mark=((idx + 1) * 8) // n_items,
                            )
                        )

            state = {"ctx_ps": None, "ctxN": {}, "drained": 0}

            def emit_scores(it):
                ci, p, blk, mode = it["ci"], it["p"], it["blk"], it["mode"]
                q0 = ci * CHUNK
                c0 = max(0, blk * 128 - q0) if mode == "causal" else 0
                it["c0"] = c0
                s_ps = sp.tile([128, 2 * CHUNK], F32, tag="scores", name="s_ps")
                es = es_pool.tile([128, 2 * CHUNK], BF16, tag="es", name="es")
                it["es"] = es
                # M<=64 matmuls double-pump on TRN2 (~2x faster per moving
                # row, near-zero weight-load cost): emit each head's score
                # block as two M=64 matmuls over disjoint sk partition halves
                for hh in range(2):
                    o = hh * D
                    for sk in range(2):
                        ks = slice(blk * 128 + sk * 64, blk * 128 + (sk + 1) * 64)
                        nc.tensor.matmul(
                            s_ps[64 * sk : 64 * sk + 64, hh * CHUNK + c0 : (hh + 1) * CHUNK],
                            kAT[p][o : o + D, ks],
                            qT[p][o : o + D, q0 + c0 : q0 + CHUNK],
                            start=True,
                            stop=True,
                        )
                if c0 == 0:
                    nc.scalar.activation(es[:], s_ps[:], Exp)
                else:
                    for hh in range(2):
                        hs0 = hh * CHUNK
                        nc.scalar.activation(
                            es[:, hs0 + c0 : hs0 + CHUNK],
                            s_ps[:, hs0 + c0 : hs0 + CHUNK],
                            Exp,
                        )
                # masking: bf16 SBUF multiplies are ~14ns on DVE (measured);
                # gpsimd software ops cost ~40x that, so everything stays DVE
                if mode == "causal":
                    for hh in range(2):
                        hs0 = hh * CHUNK
                        nc.vector.tensor_mul(
                            es[:, hs0 + c0 : hs0 + c0 + 128],
                            es[:, hs0 + c0 : hs0 + c0 + 128],
                            tri_sb[:],
                        )
                elif mode == "tile":
                    for hh in range(2):
                        hs0 = hh * CHUNK
                        nc.vector.tensor_mul(
                            es[:, hs0 : hs0 + CHUNK],
                            es[:, hs0 : hs0 + CHUNK],
                            mtiles[it["aux"]][:],
                        )

            def emit_ctx(it):
                ci, p, blk, c0 = it["ci"], it["p"], it["blk"], it["c0"]
                q0 = ci * CHUNK
                es = it["es"]
                if it["first"]:
                    state["ctx_ps"] = [
                        cp.tile([D + 1, CHUNK], F32, tag="ctx", name=f"ctx{hh}")
                        for hh in range(2)
                    ]
                ctx_ps = state["ctx_ps"]
                for hh in range(2):
                    h = 2 * p + hh
                    nc.tensor.matmul(
                        ctx_ps[hh][:, c0:],
                        va[h][:, blk * (D + 1) : (blk + 1) * (D + 1)],
                        es[:, hh * CHUNK + c0 : (hh + 1) * CHUNK],
                        start=it["first"],
                        stop=it["last"],
                    )
                while state["drained"] < it["drain_mark"] and pending:
                    drain_one()
                    state["drained"] += 1
                if it["last"]:
                    # normalize: ctxN = ctxU * (1/r); r_inv broadcast to 64
                    # partitions via a K=1 PE matmul (ones outer r_inv)
                    ctxN_p = ctxn.tile([128, CHUNK], F32R, tag="ctxN")
                    r_invs = []
                    for hh in range(2):
                        if K_BCAST == "pe":
                            r_inv = rr.tile([1, CHUNK], F32R, tag="rinv")
                            with nc.allow_low_precision(
                                reason="1/r broadcast via fp32r matmul; rounding "
                                "error is ~2^-19 relative on a softmax sum"
                            ):
                                nc.vector.reciprocal(
                                    r_inv[:], ctx_ps[hh][D : D + 1, :]
                                )
                        else:
                            r_inv = rr.tile([1, CHUNK], F32, tag="rinv")
                            nc.vector.reciprocal(r_inv[:], ctx_ps[hh][D : D + 1, :])
                        r_invs.append(r_inv)
                    for hh in range(2):
                        o = hh * D
                        if K_BCAST == "pe":
                            r_bc = op.tile([D, CHUNK], F32, tag="ctx", name="r_bc")
                            nc.tensor.matmul(
                                r_bc[:],
                                ones_sb[:],
                                r_invs[hh][:],
                                start=True,
                                stop=True,
                            )
                        else:
                            r_bc = rr.tile([D, CHUNK], F32, tag="rbc")
                            nc.gpsimd.partition_broadcast(r_bc[:], r_invs[hh][:])
                        nc.vector.tensor_mul(
                            ctxN_p[o : o + D, :], ctx_ps[hh][0:D, :], r_bc[:]
                        )
                    state["ctxN"][p] = ctxN_p
                    if p == 1:
                        pair = (state["ctxN"][0], state["ctxN"][1])
                        for sb in range(CHUNK // 128):
                            pending.extend(
                                outp_units(pair, q0, sb, fine=it["fine"])
                            )
                        state["drained"] = 0

            emit_kA(chunk_order[0])
            # scores run CTX_LAG items ahead of their exp->mask->ctx
            # consumers so the in-order PE queue never waits on Activation
            ctx_lag = int(os.environ.get("K_CTX_LAG", "2"))
            backlog = []
            for it in items:
                emit_scores(it)
                backlog.append(it)
                if len(backlog) > ctx_lag:
                    emit_ctx(backlog.pop(0))
                if it["hoist_kA"] is not None:
                    emit_kA(it["hoist_kA"])
            while backlog:
                emit_ctx(backlog.pop(0))
            while pending:
                drain_one()

        if hw_loop:
            _emit_prep()
            with tc.For_i(0, hw_loop) as _i:
                _emit_body(False)
        else:
            for _rep in range(repeat):
                _emit_body(_rep == 0)

    nc.compile()
    return nc


def prepare(key, query, value, mask, Wq, Wk, Wv, Wo, bo, build=True):
    """Host-side sharding/layout prep. Returns (nc, in_maps, gather)."""
    key = np.asarray(key, dtype=np.float32)
    query = np.asarray(query, dtype=np.float32)
    value = np.asarray(value, dtype=np.float32)
    Wq = np.asarray(Wq, dtype=np.float32)
    Wk = np.asarray(Wk, dtype=np.float32)
    Wv = np.asarray(Wv, dtype=np.float32)
    Wo = np.asarray(Wo, dtype=np.float32)
    bo = np.asarray(bo, dtype=np.float32)

    schedule, mtiles = _analyze_mask(mask)
    nc = build_nc(schedule, len(mtiles)) if build else None

    woT_all = np.ascontiguousarray(Wo.T.reshape(H, D, E))  # per head: Wo[:, cols_h].T
    bo4 = (bo / 4.0).reshape(1, E)
    mt = np.stack(mtiles).astype(BF16_NP) if mtiles else None
    tri = (
        (np.arange(128)[None, :] >= np.arange(128)[:, None])
        .astype(BF16_NP)
        .reshape(128, 128)
    )

    in_maps = []
    for c in range(N_CORES):
        b = c // 4
        h0 = 4 * (c % 4)
        hs = slice(h0, h0 + 4)
        q = query[b].reshape(S, H, D)[:, hs, :]  # [S, 4, D]
        k = key[b].reshape(S, H, D)[:, hs, :]
        v = value[b].reshape(S, H, D)[:, hs, :]
        # pair-stacked transposed layouts [2, 128, S]
        qT = np.ascontiguousarray(q.transpose(1, 2, 0).reshape(2, 2 * D, S))
        kT = np.ascontiguousarray(k.transpose(1, 2, 0).reshape(2, 2 * D, S))
        va = np.ones((4, S, D + 1), dtype=np.float32)
        va[:, :, :D] = v.transpose(1, 0, 2)
        # partition-major: [4, S, D+1] -> [4, 128, N_BLK*(D+1)]
        va = (
            va.reshape(4, N_BLK, 128, D + 1)
            .transpose(0, 2, 1, 3)
            .reshape(4, 128, N_BLK * (D + 1))
        )
        m = {
            "qT": qT.astype(BF16_NP),
            "kT": kT.astype(BF16_NP),
            "va": np.ascontiguousarray(va).astype(BF16_NP),
            "tri": tri,
            "wq": Wq,
            "wk": Wk,
            "wv": Wv,
            "woT": woT_all[h0 : h0 + 4],
            "bo4": bo4,
        }
        if mt is not None:
            m["mtiles"] = mt
        in_maps.append(m)

    def gather(results):
        out = np.empty((B, S, E), dtype=np.float32)
        for b in range(B):
            acc = results[4 * b]["out"].astype(np.float32).copy()
            for c in range(4 * b + 1, 4 * b + 4):
                acc += results[c]["out"]
            out[b] = acc
        return out

    return nc, in_maps, gather


def kernel(key, query, value, mask, Wq, Wk, Wv, Wo, bo):
    nc, in_maps, gather = prepare(key, query, value, mask, Wq, Wk, Wv, Wo, bo)
    res = run_bass_kernel_spmd(nc, in_maps, core_ids=list(range(N_CORES)))
    return gather(res.results)
